# revision 2
# baseline (speedup 1.0000x reference)
"""Masked window self-attention block (Swin-style) on 8 Trainium2 NeuronCores.

kernel(**inputs) takes FULL unsharded inputs, returns the FULL (32, 3136, 256)
float32 output. Data-parallel over batch: each core processes 4 images with a
Bass/Tile kernel (bf16 matmuls, gate-based triple softmax). Falls back to an
exact CPU implementation if the device path fails or the inputs don't match
the structural assumptions the device kernel exploits.
"""
import numpy as np

B, Hh, Ww, C = 32, 56, 56, 256
WIN, SHIFT, HEADS = 7, 3, 8
N = WIN * WIN
nH = Hh // WIN
nW = nH * (Ww // WIN)
D = C // HEADS
N_CORES = 8
IMG_PER_CORE = B // N_CORES
L = Hh * Ww


def _rel_pos_index(w):
    coords = np.stack(np.meshgrid(np.arange(w), np.arange(w), indexing="ij"))
    flat = coords.reshape(2, -1)
    rel = (flat[:, :, None] - flat[:, None, :]).transpose(1, 2, 0).copy()
    rel[..., 0] += w - 1
    rel[..., 1] += w - 1
    rel[..., 0] *= 2 * w - 1
    return rel.sum(-1)


_REL_IDX = _rel_pos_index(WIN)


def _device_ok(qkv_b, proj_b, fc2_b, norm1_g, norm1_b, norm2_g, norm2_b,
               attn_mask, sal_fg, sal_bg):
    try:
        if not (np.all(qkv_b == 0) and np.all(proj_b == 0) and np.all(fc2_b == 0)):
            return False
        if not (np.all(norm1_g == 1) and np.all(norm1_b == 0)
                and np.all(norm2_g == 1) and np.all(norm2_b == 0)):
            return False
        for m in (attn_mask, sal_fg, sal_bg):
            u = np.unique(m)
            if not np.all(np.isin(u, (np.float32(0.0), np.float32(-100.0)))):
                return False
        # sal masks must be key-only (n-replicated) and complementary
        if not np.array_equal(sal_fg, np.broadcast_to(sal_fg[:, :1, :], sal_fg.shape)):
            return False
        if not np.array_equal((sal_fg == 0) | (sal_bg == 0),
                              np.ones_like(sal_fg, dtype=bool)):
            return False
        if not np.array_equal((sal_fg == 0) & (sal_bg == 0),
                              np.zeros_like(sal_fg, dtype=bool)):
            return False
        return True
    except Exception:
        return False


def kernel(x, qkv_w, qkv_b, rpb_table, proj_w, proj_b, norm1_g, norm1_b,
           norm2_g, norm2_b, fc1_w, fc1_b, fc2_w, fc2_b,
           attn_mask, sal_fg_attn_mask, sal_bg_attn_mask):
    args = [np.asarray(a, np.float32) for a in (
        x, qkv_w, qkv_b, rpb_table, proj_w, proj_b, norm1_g, norm1_b,
        norm2_g, norm2_b, fc1_w, fc1_b, fc2_w, fc2_b,
        attn_mask, sal_fg_attn_mask, sal_bg_attn_mask)]
    (x, qkv_w, qkv_b, rpb_table, proj_w, proj_b, norm1_g, norm1_b,
     norm2_g, norm2_b, fc1_w, fc1_b, fc2_w, fc2_b,
     attn_mask, sal_fg, sal_bg) = args

    if _device_ok(qkv_b, proj_b, fc2_b, norm1_g, norm1_b, norm2_g, norm2_b,
                  attn_mask, sal_fg, sal_bg):
        try:
            return _kernel_device(x, qkv_w, rpb_table, proj_w, fc1_w, fc1_b,
                                  fc2_w, attn_mask, sal_fg)
        except Exception:
            import traceback
            traceback.print_exc()

    return _kernel_cpu(*args)


def _kernel_device(x, qkv_w, rpb_table, proj_w, fc1_w, fc1_b, fc2_w,
                   attn_mask, sal_fg):
    import wsa_host
    import wsa_driver

    consts = wsa_host.prep_consts(qkv_w, None, rpb_table, proj_w, fc1_w, fc1_b,
                                  fc2_w, attn_mask)
    run = wsa_driver.get_runner(IMG_PER_CORE)

    in_maps = []
    for c in range(N_CORES):
        i0 = c * IMG_PER_CORE
        gf = wsa_host.prep_gf(
            sal_fg[i0 * nW:(i0 + IMG_PER_CORE) * nW], IMG_PER_CORE)
        m = dict(consts)
        m["x"] = np.ascontiguousarray(
            x[i0:i0 + IMG_PER_CORE].reshape(IMG_PER_CORE * L, C))
        m["gf"] = gf
        in_maps.append(m)

    outs = run(in_maps)
    full = np.concatenate([o["out"].reshape(IMG_PER_CORE, L, C) for o in outs], 0)
    return np.ascontiguousarray(full.astype(np.float32))


# ---------------- exact CPU fallback ----------------

def _erf(x):
    try:
        from scipy.special import erf as _serf
        return _serf(x).astype(np.float32)
    except Exception:
        s = np.sign(x)
        a = np.abs(x.astype(np.float64))
        t = 1.0 / (1.0 + 0.3275911 * a)
        y = 1.0 - (((((1.061405429 * t - 1.453152027) * t) + 1.421413741) * t
                    - 0.284496736) * t + 0.254829592) * t * np.exp(-a * a)
        return (s * y).astype(np.float32)


def _layer_norm(x, g, b):
    m = x.mean(-1, keepdims=True)
    v = ((x - m) ** 2).mean(-1, keepdims=True)
    return (x - m) / np.sqrt(v + 1e-5) * g + b


def _softmax(a):
    a = a - a.max(-1, keepdims=True)
    e = np.exp(a)
    return e / e.sum(-1, keepdims=True)


def _kernel_cpu(x, qkv_w, qkv_b, rpb_table, proj_w, proj_b, norm1_g, norm1_b,
                norm2_g, norm2_b, fc1_w, fc1_b, fc2_w, fc2_b,
                attn_mask, sal_fg, sal_bg):
    outs = []
    per = B // N_CORES
    for c in range(N_CORES):
        xs = x[c * per:(c + 1) * per]
        sf = sal_fg[c * per * nW:(c + 1) * per * nW]
        sb = sal_bg[c * per * nW:(c + 1) * per * nW]
        outs.append(_block_numpy(xs, qkv_w, qkv_b, rpb_table, proj_w, proj_b,
                                 norm1_g, norm1_b, norm2_g, norm2_b,
                                 fc1_w, fc1_b, fc2_w, fc2_b, attn_mask, sf, sb))
    return np.concatenate(outs, 0)


def _block_numpy(x, qkv_w, qkv_b, rpb_table, proj_w, proj_b, norm1_g, norm1_b,
                 norm2_g, norm2_b, fc1_w, fc1_b, fc2_w, fc2_b,
                 attn_mask, sal_fg, sal_bg):
    Bb, L_, Cc = x.shape
    scale = np.float32(D ** -0.5)
    shortcut = x
    xn = _layer_norm(x, norm1_g, norm1_b).reshape(Bb, Hh, Ww, Cc)
    xs = np.roll(xn, (-SHIFT, -SHIFT), axis=(1, 2))
    xw = (xs.reshape(Bb, nH, WIN, nH, WIN, Cc).transpose(0, 1, 3, 2, 4, 5)
          .reshape(-1, N, Cc))
    B_ = xw.shape[0]
    qkv = (xw @ qkv_w.T + qkv_b).reshape(B_, N, 3, HEADS, D).transpose(2, 0, 3, 1, 4)
    q, k, v = qkv[0] * scale, qkv[1], qkv[2]
    attn = np.einsum("bhnd,bhmd->bhnm", q, k).astype(np.float32)
    rpb = rpb_table[_REL_IDX.reshape(-1)].reshape(N, N, HEADS).transpose(2, 0, 1)
    attn = attn + rpb[None]
    attn_fg = attn + sal_fg[:, None]
    attn_bg = attn + sal_bg[:, None]

    def add_shift(a):
        a = a.reshape(B_ // nW, nW, HEADS, N, N) + attn_mask[None, :, None]
        return a.reshape(B_, HEADS, N, N)

    p = _softmax(add_shift(attn))
    p_fg = _softmax(add_shift(attn_fg))
    p_bg = _softmax(add_shift(attn_bg))
    o = np.einsum("bhnm,bhmd->bhnd", p + p_fg - p_bg, v).astype(np.float32)
    o = o.transpose(0, 2, 1, 3).reshape(B_, N, Cc)
    o = o @ proj_w.T + proj_b
    xr = (o.reshape(Bb, nH, nH, WIN, WIN, Cc).transpose(0, 1, 3, 2, 4, 5)
          .reshape(Bb, Hh, Ww, Cc))
    xr = np.roll(xr, (SHIFT, SHIFT), axis=(1, 2)).reshape(Bb, L_, Cc)
    x2 = shortcut + xr
    h = _layer_norm(x2, norm2_g, norm2_b)
    h1 = h @ fc1_w.T + fc1_b
    h1 = h1 * 0.5 * (1.0 + _erf(h1 * np.float32(1.0 / np.sqrt(2.0))))
    h = h1 @ fc2_w.T + fc2_b
    return (x2 + h).astype(np.float32)
